# revision 18
# baseline (speedup 1.0000x reference)
"""Batched same-batch KNN (top-3) + fused MLP for Trainium2, 8 NeuronCores.

Strategy
--------
Host side (numpy, exact):
  * Stable-group rows of a and b by batch id. Batch g -> core g (B == 8 ==
    n_cores). Within a batch the original relative order is preserved, so
    the device's tie handling matches jax.lax.top_k.
  * Composite-distance trick: the device computes, per [Na_g, Nb_g] pair,
        m[i,j] = -(dist2(i,j) + j * 2^-11)
    via ONE K=8 fp16 matmul (4 concurrent PE row-groups). All distance
    terms are small integers split so every fp16 operand and every fp32
    partial sum is exact; the j-term rides in the LAST K row, so it is
    added to the settled integer distance. For dist2 < 8192 (the only
    region where the weight dw > 0) the composite is exactly
    representable in fp32, so top-3 selection, tie order (smallest j
    first, = jax), the gather index j, and dw are all exact.
Device side (per core, SPMD):
  * R = relu(feats_bg @ W1) in bf16 DRAM (b1 == 0 asserted host-side;
    rows 1664..2047 zeroed so don't-care indices stay harmless).
  * Per 128-row a-tile: K=8 matmul -> 2 PSUM chunks; DVE max8 per chunk
    straight from PSUM + an 16-wide merge max8 (the only DVE scans);
    dw = relu(0.5 + m/16384) (j-term shifts dw by <= 6e-5, negligible);
    j = uint32(-m*2048) & 2047; ONE dma_gather fetches all 3*128 R rows
    (wrapped int16 index layout built by a tiny DRAM bounce); the
    dw^2-weighted sum + transpose fuse into 6 bf16 PE matmuls against
    diag(dw2_k) accumulating in PSUM; then the bf16 @W2 (+3*b2) tail
    writes the fusedT output slab.
Outputs are scattered back to original row order on host; the feats_a
passthrough half of the concat is host-side assembly.
"""

import os
import numpy as np
import ml_dtypes

import concourse.bass as bass
import concourse.mybir as mybir
import concourse.tile as tile
from concourse import bacc
from concourse.bass import IndirectOffsetOnAxis
from concourse.bass_utils import run_bass_kernel_spmd
from concourse.masks import make_identity

P = 128
NPAD = 1664  # 13 * 128; covers per-batch row counts for Na=Nb=12288, B=8
NT = NPAD // P
DF = 256
TOPK = 3
FULL_SCALE = 128
RCLIP = 0.5
INV_SCALE2 = 1.0 / (FULL_SCALE * FULL_SCALE)
N_CORES = 8
HCH = NPAD // 2  # 832: distance PSUM chunk width
QCH = NPAD // 4  # 416: per-PE-row-group column span
RROWS = 2048  # R table rows (pow2 so j & 2047 is always in bounds)

_PROGRAM_CACHE = {}


def _build_program():
    """Build the SPMD Bass program (identical on all 8 cores)."""
    nc = bacc.Bacc("TRN2", target_bir_lowering=False, debug=False)
    f32 = mybir.dt.float32
    f16 = mybir.dt.float16
    bf16 = mybir.dt.bfloat16
    i16 = mybir.dt.int16
    u32 = mybir.dt.uint32

    uaT = nc.dram_tensor("uaT", [8, NPAD], f16, kind="ExternalInput").ap()
    vbT = nc.dram_tensor("vbT", [8, NPAD], f16, kind="ExternalInput").ap()
    fbTb = nc.dram_tensor("fbTb", [DF, NPAD], bf16, kind="ExternalInput").ap()
    w1b = nc.dram_tensor("w1b", [DF, DF], bf16, kind="ExternalInput").ap()
    w2b = nc.dram_tensor("w2b", [DF, DF], bf16, kind="ExternalInput").ap()
    b2c3 = nc.dram_tensor("b2c3", [P, 2], f32, kind="ExternalInput").ap()
    fusedT = nc.dram_tensor("fusedT", [DF, NPAD], f32, kind="ExternalOutput").ap()

    with tile.TileContext(nc) as tc:
        with (
            tc.tile_pool(name="const", bufs=1) as cpool,
            tc.tile_pool(name="dram", bufs=1, space="DRAM") as dpool_dram,
        ):
            # R-phase dependencies first: their loads gate rtab, which gates
            # every gather. fbT arrives in 416-col pieces so MM1 tile 0 can
            # start as soon as the first piece lands.
            fbT0 = cpool.tile([P, NPAD], bf16)
            fbT1 = cpool.tile([P, NPAD], bf16)
            w1k0 = cpool.tile([P, DF], bf16)
            w1k1 = cpool.tile([P, DF], bf16)
            nc.sync.dma_start(fbT0[:, 0:QCH], fbTb[0:P, 0:QCH])
            nc.sync.dma_start(fbT1[:, 0:QCH], fbTb[P : 2 * P, 0:QCH])
            nc.sync.dma_start(w1k0[:], w1b[0:P, :])
            nc.sync.dma_start(w1k1[:], w1b[P : 2 * P, :])
            for q in range(1, 4):
                qs = slice(q * QCH, (q + 1) * QCH)
                nc.sync.dma_start(fbT0[:, qs], fbTb[0:P, qs])
                nc.sync.dma_start(fbT1[:, qs], fbTb[P : 2 * P, qs])
            ua4 = cpool.tile([8, NPAD], f16)
            nc.sync.dma_start(ua4[:], uaT[:])
            vb4 = cpool.tile([8, NPAD], f16)
            nc.sync.dma_start(vb4[:], vbT[:])
            w2k0 = cpool.tile([P, DF], bf16)
            nc.sync.dma_start(w2k0[:], w2b[0:P, :])
            w2k1 = cpool.tile([P, DF], bf16)
            nc.sync.dma_start(w2k1[:], w2b[P : 2 * P, :])
            b2s = cpool.tile([P, 2], f32)
            nc.sync.dma_start(b2s[:], b2c3[:])
            identb = cpool.tile([P, P], bf16)
            make_identity(nc, identb[:])
            halfcol = cpool.tile([P, 1], f32)
            nc.gpsimd.memset(halfcol[:], RCLIP)
            zrow = cpool.tile([P, DF], bf16)
            nc.gpsimd.memset(zrow[:], 0.0)

            rtab = dpool_dram.tile([RROWS, DF], bf16)
            # zero the don't-care tail rows 1664..2047 right away
            for t in range(NT, RROWS // P):
                nc.sync.dma_start(rtab[bass.ts(t, P), :], zrow[:])

            # ---- Phase D pools (f_pool doubles as the R-phase MM1 PSUM so
            # the early top-k tiles can overlap the R phase)
            with (
                tc.tile_pool(name="dps", bufs=2, space="PSUM") as d_pool,
                tc.tile_pool(name="tps", bufs=2, space="PSUM") as t_pool,
                tc.tile_pool(name="fps", bufs=2, space="PSUM") as f_pool,
                tc.tile_pool(name="small", bufs=8) as s_pool,
                tc.tile_pool(name="diag", bufs=8) as diag_pool,
                tc.tile_pool(name="gat", bufs=8) as g_pool,
                tc.tile_pool(name="acc", bufs=2) as a_pool,
                tc.tile_pool(name="outp", bufs=2) as o_pool,
                tc.tile_pool(name="rsb", bufs=6) as r_pool,
            ):
                state = {}

                def r_phase():
                    for t in range(NT):
                        sl = bass.ts(t, P)
                        pool = f_pool if t % 2 == 0 else t_pool
                        psR = pool.tile([P, DF], f32, tag="fus" if t % 2 == 0 else "accT")
                        # b1 asserted zero host-side (numpy fallback otherwise)
                        nc.tensor.matmul(
                            psR[:], lhsT=fbT0[:, sl], rhs=w1k0[:],
                            start=True, stop=False,
                        )
                        nc.tensor.matmul(
                            psR[:], lhsT=fbT1[:, sl], rhs=w1k1[:],
                            start=False, stop=True,
                        )
                        rt = r_pool.tile([P, DF], bf16)
                        nc.scalar.activation(
                            rt[:], psR[:], mybir.ActivationFunctionType.Relu
                        )
                        nc.sync.dma_start(rtab[sl, :], rt[:])


                def topk_and_gather(t):
                    sl = bass.ts(t, P)
                    vals = s_pool.tile([P, 16], f32, tag="vals")
                    for c in range(2):
                        dps = d_pool.tile([P, HCH], f32)
                        # two matmuls per chunk (512 + 320 cols), each within
                        # one PSUM bank; 4 PE row-groups across the two chunks
                        for l0, l1 in ((0, 512), (512, HCH)):
                            nc.tensor.matmul(
                                dps[:, l0:l1],
                                lhsT=ua4[:, sl],
                                rhs=vb4[:, c * HCH + l0 : c * HCH + l1],
                                start=True,
                                stop=True,
                            )
                        # top-8 of this chunk, straight from PSUM
                        nc.vector.max(out=vals[:, 8 * c : 8 * (c + 1)], in_=dps[:])
                    vals3 = s_pool.tile([P, 8], f32, tag="vals3")
                    nc.vector.max(out=vals3[:], in_=vals[:])
                    # dw = relu(0.5 + m/16384); j-term shifts it by <=6e-5
                    dw = s_pool.tile([P, TOPK], f32, tag="dw")
                    nc.scalar.activation(
                        dw[:],
                        vals3[:, 0:TOPK],
                        mybir.ActivationFunctionType.Relu,
                        bias=halfcol[:],
                        scale=INV_SCALE2,
                    )
                    # j = uint32(-m * 2048) & 2047 — issued before the diag
                    # builds so the gathers unblock as early as possible
                    c32 = s_pool.tile([P, TOPK], u32, tag="c32")
                    nc.scalar.mul(c32[:], vals3[:, 0:TOPK], -2048.0)
                    jand = s_pool.tile([P, TOPK], u32, tag="jand")
                    nc.vector.tensor_scalar(
                        out=jand[:],
                        in0=c32[:],
                        scalar1=2047,
                        scalar2=None,
                        op0=mybir.AluOpType.bitwise_and,
                    )
                    dw2 = s_pool.tile([P, TOPK], f32, tag="dw2")
                    nc.scalar.activation(
                        dw2[:], dw[:], mybir.ActivationFunctionType.Square
                    )
                    # diag_k = dw2[:, k] on the diagonal (bf16), PE rhs operands
                    diag = diag_pool.tile([P, TOPK * P], bf16)
                    for k in range(TOPK):
                        nc.scalar.mul(
                            diag[:, k * P : (k + 1) * P], identb[:], dw2[:, k : k + 1]
                        )
                    state[t] = (diag, jand)

                def gather_tile(t):
                    # 3 indirect gathers (SWDGE descriptors spread across all
                    # 16 DMA engines): g[p, k*DF:(k+1)*DF] = R[jand[p, k]].
                    # MUST be issued after the rtab writes in program order.
                    diag, jand = state.pop(t)
                    g = g_pool.tile([P, TOPK * DF], bf16)
                    for k in range(TOPK):
                        nc.gpsimd.indirect_dma_start(
                            out=g[:, k * DF : (k + 1) * DF],
                            out_offset=None,
                            in_=rtab[:],
                            in_offset=IndirectOffsetOnAxis(
                                ap=jand[:, k : k + 1], axis=0
                            ),
                        )
                    state[t] = (diag, g)

                def mlp_tile(t):
                    sl = bass.ts(t, P)
                    diag, g = state.pop(t)
                    # accT[i, p] = sum_k dw2_k[p] * g_k[p, i]  (scaled transpose,
                    # PSUM-accumulated over k; m indexes the two i-halves)
                    accT = t_pool.tile([P, DF], f32, tag="accT")
                    for m in range(2):
                        for k in range(TOPK):
                            nc.tensor.matmul(
                                accT[:, m * P : (m + 1) * P],
                                lhsT=g[:, k * DF + m * P : k * DF + (m + 1) * P],
                                rhs=diag[:, k * P : (k + 1) * P],
                                start=(k == 0),
                                stop=(k == TOPK - 1),
                            )
                    accTs = a_pool.tile([P, DF], bf16)
                    nc.scalar.copy(accTs[:], accT[:])
                    # fusedT[o, p] = sum_i W2[i, o] * accT[i, p]  (+ 3*b2[o])
                    fus = f_pool.tile([P, DF], f32, tag="fus")
                    for mo in range(2):
                        osl = slice(mo * P, (mo + 1) * P)
                        nc.tensor.matmul(
                            fus[:, osl],
                            lhsT=w2k0[:, osl],
                            rhs=accTs[:, 0:P],
                            start=True,
                            stop=False,
                        )
                        nc.tensor.matmul(
                            fus[:, osl],
                            lhsT=w2k1[:, osl],
                            rhs=accTs[:, P : 2 * P],
                            start=False,
                            stop=True,
                        )
                    oT = o_pool.tile([P, DF], f32)
                    for mo in range(2):
                        osl = slice(mo * P, (mo + 1) * P)
                        nc.vector.tensor_scalar(
                            out=oT[:, osl],
                            in0=fus[:, osl],
                            scalar1=b2s[:, mo : mo + 1],
                            scalar2=None,
                            op0=mybir.AluOpType.add,
                        )
                        nc.sync.dma_start(fusedT[osl, sl], oT[:, osl])

                SKEW = 3
                # R phase first: MM1 completion gates every gather, so no
                # other PE work may precede it
                r_phase()
                for t in range(NT + SKEW):
                    if t < NT:
                        topk_and_gather(t)
                        gather_tile(t)
                    if t >= SKEW:
                        mlp_tile(t - SKEW)
    nc.compile()
    return nc


def get_program():
    if "nc" not in _PROGRAM_CACHE:
        _PROGRAM_CACHE["nc"] = _build_program()
    return _PROGRAM_CACHE["nc"]


def _host_prep(batch_a, coords_a, batch_b, coords_b, feats_b, W1, b1, W2, b2):
    """Group by batch, build per-core input arrays. Returns (in_maps, meta)."""
    pa = np.argsort(batch_a, kind="stable")
    pb = np.argsort(batch_b, kind="stable")
    ca = np.bincount(batch_a, minlength=N_CORES)
    cb = np.bincount(batch_b, minlength=N_CORES)
    oa = np.concatenate([[0], np.cumsum(ca)])
    ob = np.concatenate([[0], np.cumsum(cb)])

    w1bf = W1.astype(ml_dtypes.bfloat16)
    w2bf = W2.astype(ml_dtypes.bfloat16)
    b2c3 = np.ascontiguousarray((3.0 * b2).astype(np.float32).reshape(2, P).T)
    jrow = -(np.arange(NPAD).astype(np.float64) * 2.0**-11)

    in_maps = []
    meta = []
    for g in range(N_CORES):
        a_idx = pa[oa[g] : oa[g + 1]]
        b_idx = pb[ob[g] : ob[g + 1]]
        na, nb = len(a_idx), len(b_idx)
        if na > NPAD or nb > NPAD or (0 < nb < TOPK):
            return None, None  # shapes outside the compiled envelope -> fallback
        xa = (coords_a[a_idx] // 16).astype(np.int64)
        xb = (coords_b[b_idx] // 16).astype(np.int64)
        if xa.size and (xa.min() < 0 or xa.max() > 127):
            return None, None  # outside the exact-f16 envelope -> fallback
        if xb.size and (xb.min() < 0 or xb.max() > 127):
            return None, None

        # composite-distance operands (all values exactly representable in f16)
        A = np.square(xa).sum(1)  # [na], <= 3*127^2 = 48387
        B = np.square(xb).sum(1)
        uaT = np.zeros((8, NPAD), dtype=np.float16)
        uaT[2, :] = 1.0
        uaT[3, :] = 1.0
        uaT[7, :] = 1.0
        if na > 0:
            uaT[0, :na] = -(A // 256)
            uaT[1, :na] = -(A % 256)
            uaT[4:7, :na] = (2 * xa).T
            if na < NPAD:
                uaT[:, na:] = uaT[:, :1]  # pad a-rows: copy of row 0, dropped
        vbT = np.zeros((8, NPAD), dtype=np.float16)
        vbT[0, :] = 256.0
        vbT[1, :] = 1.0
        vbT[2, :] = -256.0 * 200.0  # pad cols: dist2 ~ 51200, never in top-3
        vbT[7, :] = jrow
        if nb > 0:
            vbT[2, :nb] = -256.0 * (B // 256)
            vbT[3, :nb] = -(B % 256)
            vbT[4:7, :nb] = xb.T

        fbT = np.zeros((DF, NPAD), dtype=ml_dtypes.bfloat16)
        if nb > 0:
            fbT[:, :nb] = feats_b[b_idx].astype(ml_dtypes.bfloat16).T

        in_maps.append(
            {
                "uaT": uaT,
                "vbT": vbT,
                "fbTb": fbT,
                "w1b": w1bf,
                "w2b": w2bf,
                "b2c3": b2c3,
            }
        )
        meta.append((a_idx, na, nb))
    return in_maps, meta


def _reference_numpy(batch_a, coords_a, feats_a, batch_b, coords_b, feats_b,
                     W1, b1, W2, b2):
    """Exact numpy fallback (mirrors reference.py) for out-of-envelope data."""
    xa = (coords_a // 16).astype(np.float32)
    xb = (coords_b // 16).astype(np.float32)
    d = (
        np.square(xa).sum(1)[:, None]
        + np.square(xb).sum(1)[None, :]
        - 2.0 * (xa @ xb.T)
    )
    d = np.clip(d, 0.0, None) / (FULL_SCALE**2)
    same = batch_a[:, None] == batch_b[None, :]
    d = np.where(same, d, np.inf)
    idx = np.argsort(d, axis=1, kind="stable")[:, :TOPK]
    dv = np.take_along_axis(d, idx, axis=1)
    dwt = RCLIP - np.clip(dv, 0.0, RCLIP)
    b_f = feats_b[idx] * dwt[..., None]
    h = np.maximum(b_f @ W1 + b1, 0.0) * dwt[..., None]
    fused = (h @ W2 + b2).sum(axis=1)
    return np.concatenate([feats_a, fused], axis=1).astype(np.float32)


def _ensure_ntff_hook():
    """Install the axon NTFF profile hook (missing antenv.axon_hooks shim)."""
    import sys
    import types

    if "antenv.axon_hooks" in sys.modules:
        return
    try:
        from trn_agent_boot.trn_boot import _ntff_profile_via_ctypes

        hook = _ntff_profile_via_ctypes("/opt/axon/libaxon_pjrt.so")
    except Exception:
        hook = None
    mod = types.ModuleType("antenv.axon_hooks")
    _state = {"hook": hook}
    mod.get_axon_ntff_profile_hook = lambda: _state["hook"]

    def _set(h):
        _state["hook"] = h

    mod.set_axon_ntff_profile_hook = _set
    sys.modules["antenv.axon_hooks"] = mod


def kernel(batch_a, coords_a, feats_a, batch_b, coords_b, feats_b, W1, b1, W2, b2):
    batch_a = np.asarray(batch_a)
    coords_a = np.asarray(coords_a)
    feats_a = np.asarray(feats_a, dtype=np.float32)
    batch_b = np.asarray(batch_b)
    coords_b = np.asarray(coords_b)
    feats_b = np.asarray(feats_b, dtype=np.float32)
    W1 = np.asarray(W1, dtype=np.float32)
    b1 = np.asarray(b1, dtype=np.float32)
    W2 = np.asarray(W2, dtype=np.float32)
    b2 = np.asarray(b2, dtype=np.float32)

    if np.any(b1 != 0.0):
        # device pipeline folds dw through relu; exact only for b1 == 0
        return _reference_numpy(
            batch_a, coords_a, feats_a, batch_b, coords_b, feats_b, W1, b1, W2, b2
        )

    in_maps, meta = _host_prep(
        batch_a, coords_a, batch_b, coords_b, feats_b, W1, b1, W2, b2
    )
    if in_maps is None:
        return _reference_numpy(
            batch_a, coords_a, feats_a, batch_b, coords_b, feats_b, W1, b1, W2, b2
        )

    nc = get_program()
    trace = bool(int(os.environ.get("KERNEL_TRACE", "0")))
    if trace:
        _ensure_ntff_hook()
    res = run_bass_kernel_spmd(
        nc, in_maps, core_ids=list(range(N_CORES)), trace=trace
    )
    kernel.last_results = res

    fused = np.zeros((len(batch_a), DF), dtype=np.float32)
    for g in range(N_CORES):
        a_idx, na, nb = meta[g]
        if na == 0:
            continue
        out_g = res.results[g]["fusedT"]  # [DF, NPAD]
        if nb == 0:
            # reference: dw=0 rows -> h=0 -> fused = 3*b2
            fused[a_idx] = 3.0 * b2
        else:
            fused[a_idx] = out_g[:, :na].T
    return np.concatenate([feats_a, fused], axis=1)


# revision 19
# speedup vs baseline: 1.0428x; 1.0428x over previous
"""Batched same-batch KNN (top-3) + fused MLP for Trainium2, 8 NeuronCores.

Strategy
--------
Host side (numpy, exact):
  * Stable-group rows of a and b by batch id. Batch g -> core g (B == 8 ==
    n_cores). Within a batch the original relative order is preserved, so
    the device's tie handling matches jax.lax.top_k.
  * Composite-distance trick: the device computes, per [Na_g, Nb_g] pair,
        m[i,j] = -(dist2(i,j) + j * 2^-11)
    via ONE K=8 fp16 matmul (4 concurrent PE row-groups). All distance
    terms are small integers split so every fp16 operand and every fp32
    partial sum is exact; the j-term rides in the LAST K row, so it is
    added to the settled integer distance. For dist2 < 8192 (the only
    region where the weight dw > 0) the composite is exactly
    representable in fp32, so top-3 selection, tie order (smallest j
    first, = jax), the gather index j, and dw are all exact.
Device side (per core, SPMD):
  * R = relu(feats_bg @ W1) in bf16 DRAM (b1 == 0 asserted host-side;
    rows 1664..2047 zeroed so don't-care indices stay harmless).
  * Per 128-row a-tile: K=8 matmul -> 2 PSUM chunks; DVE max8 per chunk
    straight from PSUM + an 16-wide merge max8 (the only DVE scans);
    dw = relu(0.5 + m/16384) (j-term shifts dw by <= 6e-5, negligible);
    j = uint32(-m*2048) & 2047; ONE dma_gather fetches all 3*128 R rows
    (wrapped int16 index layout built by a tiny DRAM bounce); the
    dw^2-weighted sum + transpose fuse into 6 bf16 PE matmuls against
    diag(dw2_k) accumulating in PSUM; then the bf16 @W2 (+3*b2) tail
    writes the fusedT output slab.
Outputs are scattered back to original row order on host; the feats_a
passthrough half of the concat is host-side assembly.
"""

import os
import numpy as np
import ml_dtypes

import concourse.bass as bass
import concourse.mybir as mybir
import concourse.tile as tile
from concourse import bacc
from concourse.bass import IndirectOffsetOnAxis
from concourse.bass_utils import run_bass_kernel_spmd
from concourse.masks import make_identity

P = 128
NPAD = 1664  # 13 * 128; covers per-batch row counts for Na=Nb=12288, B=8
NT = NPAD // P
DF = 256
TOPK = 3
FULL_SCALE = 128
RCLIP = 0.5
INV_SCALE2 = 1.0 / (FULL_SCALE * FULL_SCALE)
N_CORES = 8
HCH = NPAD // 2  # 832: distance PSUM chunk width
QCH = NPAD // 4  # 416: per-PE-row-group column span
RROWS = 2048  # R table rows (pow2 so j & 2047 is always in bounds)

_PROGRAM_CACHE = {}


def _build_program():
    """Build the SPMD Bass program (identical on all 8 cores)."""
    nc = bacc.Bacc("TRN2", target_bir_lowering=False, debug=False)
    f32 = mybir.dt.float32
    f16 = mybir.dt.float16
    bf16 = mybir.dt.bfloat16
    i16 = mybir.dt.int16
    u32 = mybir.dt.uint32

    uaT = nc.dram_tensor("uaT", [8, NPAD], f16, kind="ExternalInput").ap()
    vbT = nc.dram_tensor("vbT", [8, NPAD], f16, kind="ExternalInput").ap()
    fbTb = nc.dram_tensor("fbTb", [DF, NPAD], bf16, kind="ExternalInput").ap()
    w1b = nc.dram_tensor("w1b", [DF, DF], bf16, kind="ExternalInput").ap()
    w2b = nc.dram_tensor("w2b", [DF, DF], bf16, kind="ExternalInput").ap()
    b2c3 = nc.dram_tensor("b2c3", [P, 2], f32, kind="ExternalInput").ap()
    fusedT = nc.dram_tensor("fusedT", [DF, NPAD], f32, kind="ExternalOutput").ap()

    with tile.TileContext(nc) as tc:
        with (
            tc.tile_pool(name="const", bufs=1) as cpool,
            tc.tile_pool(name="dram", bufs=1, space="DRAM") as dpool_dram,
        ):
            # R-phase dependencies first: their loads gate rtab, which gates
            # every gather
            fbT0 = cpool.tile([P, NPAD], bf16)
            nc.sync.dma_start(fbT0[:], fbTb[0:P, :])
            fbT1 = cpool.tile([P, NPAD], bf16)
            nc.sync.dma_start(fbT1[:], fbTb[P : 2 * P, :])
            w1k0 = cpool.tile([P, DF], bf16)
            nc.sync.dma_start(w1k0[:], w1b[0:P, :])
            w1k1 = cpool.tile([P, DF], bf16)
            nc.sync.dma_start(w1k1[:], w1b[P : 2 * P, :])
            ua4 = cpool.tile([8, NPAD], f16)
            nc.sync.dma_start(ua4[:], uaT[:])
            vb4 = cpool.tile([8, NPAD], f16)
            nc.sync.dma_start(vb4[:], vbT[:])
            w2k0 = cpool.tile([P, DF], bf16)
            nc.sync.dma_start(w2k0[:], w2b[0:P, :])
            w2k1 = cpool.tile([P, DF], bf16)
            nc.sync.dma_start(w2k1[:], w2b[P : 2 * P, :])
            b2s = cpool.tile([P, 2], f32)
            nc.sync.dma_start(b2s[:], b2c3[:])
            identb = cpool.tile([P, P], bf16)
            make_identity(nc, identb[:])
            halfcol = cpool.tile([P, 1], f32)
            nc.gpsimd.memset(halfcol[:], RCLIP)
            zrow = cpool.tile([P, DF], bf16)
            nc.gpsimd.memset(zrow[:], 0.0)

            rtab = dpool_dram.tile([RROWS, DF], bf16)
            # zero the don't-care tail rows 1664..2047 right away
            for t in range(NT, RROWS // P):
                nc.sync.dma_start(rtab[bass.ts(t, P), :], zrow[:])

            # ---- Phase D pools (f_pool doubles as the R-phase MM1 PSUM so
            # the early top-k tiles can overlap the R phase)
            with (
                tc.tile_pool(name="dps", bufs=2, space="PSUM") as d_pool,
                tc.tile_pool(name="tps", bufs=2, space="PSUM") as t_pool,
                tc.tile_pool(name="fps", bufs=2, space="PSUM") as f_pool,
                tc.tile_pool(name="small", bufs=8) as s_pool,
                tc.tile_pool(name="diag", bufs=8) as diag_pool,
                tc.tile_pool(name="gat", bufs=8) as g_pool,
                tc.tile_pool(name="acc", bufs=2) as a_pool,
                tc.tile_pool(name="outp", bufs=2) as o_pool,
                tc.tile_pool(name="rsb", bufs=6) as r_pool,
            ):
                state = {}

                def r_phase():
                    for t in range(NT):
                        sl = bass.ts(t, P)
                        pool = f_pool if t % 2 == 0 else t_pool
                        psR = pool.tile([P, DF], f32, tag="fus" if t % 2 == 0 else "accT")
                        # b1 asserted zero host-side (numpy fallback otherwise)
                        nc.tensor.matmul(
                            psR[:], lhsT=fbT0[:, sl], rhs=w1k0[:],
                            start=True, stop=False,
                        )
                        nc.tensor.matmul(
                            psR[:], lhsT=fbT1[:, sl], rhs=w1k1[:],
                            start=False, stop=True,
                        )
                        rt = r_pool.tile([P, DF], bf16)
                        nc.scalar.activation(
                            rt[:], psR[:], mybir.ActivationFunctionType.Relu
                        )
                        nc.sync.dma_start(rtab[sl, :], rt[:])


                def topk_and_gather(t):
                    sl = bass.ts(t, P)
                    vals = s_pool.tile([P, 16], f32, tag="vals")
                    for c in range(2):
                        dps = d_pool.tile([P, HCH], f32)
                        # two matmuls per chunk (512 + 320 cols), each within
                        # one PSUM bank; 4 PE row-groups across the two chunks
                        for l0, l1 in ((0, 512), (512, HCH)):
                            nc.tensor.matmul(
                                dps[:, l0:l1],
                                lhsT=ua4[:, sl],
                                rhs=vb4[:, c * HCH + l0 : c * HCH + l1],
                                start=True,
                                stop=True,
                            )
                        # top-8 of this chunk, straight from PSUM
                        nc.vector.max(out=vals[:, 8 * c : 8 * (c + 1)], in_=dps[:])
                    vals3 = s_pool.tile([P, 8], f32, tag="vals3")
                    nc.vector.max(out=vals3[:], in_=vals[:])
                    # dw = relu(0.5 + m/16384); j-term shifts it by <=6e-5
                    dw = s_pool.tile([P, TOPK], f32, tag="dw")
                    nc.scalar.activation(
                        dw[:],
                        vals3[:, 0:TOPK],
                        mybir.ActivationFunctionType.Relu,
                        bias=halfcol[:],
                        scale=INV_SCALE2,
                    )
                    # j = uint32(-m * 2048) & 2047 — issued before the diag
                    # builds so the gathers unblock as early as possible
                    c32 = s_pool.tile([P, TOPK], u32, tag="c32")
                    nc.scalar.mul(c32[:], vals3[:, 0:TOPK], -2048.0)
                    jand = s_pool.tile([P, TOPK], u32, tag="jand")
                    nc.vector.tensor_scalar(
                        out=jand[:],
                        in0=c32[:],
                        scalar1=2047,
                        scalar2=None,
                        op0=mybir.AluOpType.bitwise_and,
                    )
                    dw2 = s_pool.tile([P, TOPK], f32, tag="dw2")
                    nc.scalar.activation(
                        dw2[:], dw[:], mybir.ActivationFunctionType.Square
                    )
                    # diag_k = dw2[:, k] on the diagonal (bf16), PE rhs operands
                    diag = diag_pool.tile([P, TOPK * P], bf16)
                    for k in range(TOPK):
                        nc.scalar.mul(
                            diag[:, k * P : (k + 1) * P], identb[:], dw2[:, k : k + 1]
                        )
                    state[t] = (diag, jand)

                def gather_tile(t):
                    # 3 indirect gathers (SWDGE descriptors spread across all
                    # 16 DMA engines): g[p, k*DF:(k+1)*DF] = R[jand[p, k]].
                    # MUST be issued after the rtab writes in program order.
                    diag, jand = state.pop(t)
                    g = g_pool.tile([P, TOPK * DF], bf16)
                    for k in range(TOPK):
                        nc.gpsimd.indirect_dma_start(
                            out=g[:, k * DF : (k + 1) * DF],
                            out_offset=None,
                            in_=rtab[:],
                            in_offset=IndirectOffsetOnAxis(
                                ap=jand[:, k : k + 1], axis=0
                            ),
                        )
                    state[t] = (diag, g)

                def mlp_tile(t):
                    sl = bass.ts(t, P)
                    diag, g = state.pop(t)
                    # accT[i, p] = sum_k dw2_k[p] * g_k[p, i]  (scaled transpose,
                    # PSUM-accumulated over k; m indexes the two i-halves)
                    accT = t_pool.tile([P, DF], f32, tag="accT")
                    for m in range(2):
                        for k in range(TOPK):
                            nc.tensor.matmul(
                                accT[:, m * P : (m + 1) * P],
                                lhsT=g[:, k * DF + m * P : k * DF + (m + 1) * P],
                                rhs=diag[:, k * P : (k + 1) * P],
                                start=(k == 0),
                                stop=(k == TOPK - 1),
                            )
                    accTs = a_pool.tile([P, DF], bf16)
                    nc.scalar.copy(accTs[:], accT[:])
                    # fusedT[o, p] = sum_i W2[i, o] * accT[i, p]  (+ 3*b2[o])
                    fus = f_pool.tile([P, DF], f32, tag="fus")
                    for mo in range(2):
                        osl = slice(mo * P, (mo + 1) * P)
                        nc.tensor.matmul(
                            fus[:, osl],
                            lhsT=w2k0[:, osl],
                            rhs=accTs[:, 0:P],
                            start=True,
                            stop=False,
                        )
                        nc.tensor.matmul(
                            fus[:, osl],
                            lhsT=w2k1[:, osl],
                            rhs=accTs[:, P : 2 * P],
                            start=False,
                            stop=True,
                        )
                    oT = o_pool.tile([P, DF], f32)
                    for mo in range(2):
                        osl = slice(mo * P, (mo + 1) * P)
                        nc.vector.tensor_scalar(
                            out=oT[:, osl],
                            in0=fus[:, osl],
                            scalar1=b2s[:, mo : mo + 1],
                            scalar2=None,
                            op0=mybir.AluOpType.add,
                        )
                        nc.sync.dma_start(fusedT[osl, sl], oT[:, osl])

                SKEW = 3
                # R phase first: MM1 completion gates every gather, so no
                # other PE work may precede it
                r_phase()
                for t in range(NT + SKEW):
                    if t < NT:
                        topk_and_gather(t)
                        gather_tile(t)
                    if t >= SKEW:
                        mlp_tile(t - SKEW)
    nc.compile()
    return nc


def get_program():
    if "nc" not in _PROGRAM_CACHE:
        _PROGRAM_CACHE["nc"] = _build_program()
    return _PROGRAM_CACHE["nc"]


def _host_prep(batch_a, coords_a, batch_b, coords_b, feats_b, W1, b1, W2, b2):
    """Group by batch, build per-core input arrays. Returns (in_maps, meta)."""
    pa = np.argsort(batch_a, kind="stable")
    pb = np.argsort(batch_b, kind="stable")
    ca = np.bincount(batch_a, minlength=N_CORES)
    cb = np.bincount(batch_b, minlength=N_CORES)
    oa = np.concatenate([[0], np.cumsum(ca)])
    ob = np.concatenate([[0], np.cumsum(cb)])

    w1bf = W1.astype(ml_dtypes.bfloat16)
    w2bf = W2.astype(ml_dtypes.bfloat16)
    b2c3 = np.ascontiguousarray((3.0 * b2).astype(np.float32).reshape(2, P).T)
    jrow = -(np.arange(NPAD).astype(np.float64) * 2.0**-11)

    in_maps = []
    meta = []
    for g in range(N_CORES):
        a_idx = pa[oa[g] : oa[g + 1]]
        b_idx = pb[ob[g] : ob[g + 1]]
        na, nb = len(a_idx), len(b_idx)
        if na > NPAD or nb > NPAD or (0 < nb < TOPK):
            return None, None  # shapes outside the compiled envelope -> fallback
        xa = (coords_a[a_idx] // 16).astype(np.int64)
        xb = (coords_b[b_idx] // 16).astype(np.int64)
        if xa.size and (xa.min() < 0 or xa.max() > 127):
            return None, None  # outside the exact-f16 envelope -> fallback
        if xb.size and (xb.min() < 0 or xb.max() > 127):
            return None, None

        # composite-distance operands (all values exactly representable in f16)
        A = np.square(xa).sum(1)  # [na], <= 3*127^2 = 48387
        B = np.square(xb).sum(1)
        uaT = np.zeros((8, NPAD), dtype=np.float16)
        uaT[2, :] = 1.0
        uaT[3, :] = 1.0
        uaT[7, :] = 1.0
        if na > 0:
            uaT[0, :na] = -(A // 256)
            uaT[1, :na] = -(A % 256)
            uaT[4:7, :na] = (2 * xa).T
            if na < NPAD:
                uaT[:, na:] = uaT[:, :1]  # pad a-rows: copy of row 0, dropped
        vbT = np.zeros((8, NPAD), dtype=np.float16)
        vbT[0, :] = 256.0
        vbT[1, :] = 1.0
        vbT[2, :] = -256.0 * 200.0  # pad cols: dist2 ~ 51200, never in top-3
        vbT[7, :] = jrow
        if nb > 0:
            vbT[2, :nb] = -256.0 * (B // 256)
            vbT[3, :nb] = -(B % 256)
            vbT[4:7, :nb] = xb.T

        fbT = np.zeros((DF, NPAD), dtype=ml_dtypes.bfloat16)
        if nb > 0:
            fbT[:, :nb] = feats_b[b_idx].astype(ml_dtypes.bfloat16).T

        in_maps.append(
            {
                "uaT": uaT,
                "vbT": vbT,
                "fbTb": fbT,
                "w1b": w1bf,
                "w2b": w2bf,
                "b2c3": b2c3,
            }
        )
        meta.append((a_idx, na, nb))
    return in_maps, meta


def _reference_numpy(batch_a, coords_a, feats_a, batch_b, coords_b, feats_b,
                     W1, b1, W2, b2):
    """Exact numpy fallback (mirrors reference.py) for out-of-envelope data."""
    xa = (coords_a // 16).astype(np.float32)
    xb = (coords_b // 16).astype(np.float32)
    d = (
        np.square(xa).sum(1)[:, None]
        + np.square(xb).sum(1)[None, :]
        - 2.0 * (xa @ xb.T)
    )
    d = np.clip(d, 0.0, None) / (FULL_SCALE**2)
    same = batch_a[:, None] == batch_b[None, :]
    d = np.where(same, d, np.inf)
    idx = np.argsort(d, axis=1, kind="stable")[:, :TOPK]
    dv = np.take_along_axis(d, idx, axis=1)
    dwt = RCLIP - np.clip(dv, 0.0, RCLIP)
    b_f = feats_b[idx] * dwt[..., None]
    h = np.maximum(b_f @ W1 + b1, 0.0) * dwt[..., None]
    fused = (h @ W2 + b2).sum(axis=1)
    return np.concatenate([feats_a, fused], axis=1).astype(np.float32)


def _ensure_ntff_hook():
    """Install the axon NTFF profile hook (missing antenv.axon_hooks shim)."""
    import sys
    import types

    if "antenv.axon_hooks" in sys.modules:
        return
    try:
        from trn_agent_boot.trn_boot import _ntff_profile_via_ctypes

        hook = _ntff_profile_via_ctypes("/opt/axon/libaxon_pjrt.so")
    except Exception:
        hook = None
    mod = types.ModuleType("antenv.axon_hooks")
    _state = {"hook": hook}
    mod.get_axon_ntff_profile_hook = lambda: _state["hook"]

    def _set(h):
        _state["hook"] = h

    mod.set_axon_ntff_profile_hook = _set
    sys.modules["antenv.axon_hooks"] = mod


def kernel(batch_a, coords_a, feats_a, batch_b, coords_b, feats_b, W1, b1, W2, b2):
    batch_a = np.asarray(batch_a)
    coords_a = np.asarray(coords_a)
    feats_a = np.asarray(feats_a, dtype=np.float32)
    batch_b = np.asarray(batch_b)
    coords_b = np.asarray(coords_b)
    feats_b = np.asarray(feats_b, dtype=np.float32)
    W1 = np.asarray(W1, dtype=np.float32)
    b1 = np.asarray(b1, dtype=np.float32)
    W2 = np.asarray(W2, dtype=np.float32)
    b2 = np.asarray(b2, dtype=np.float32)

    if np.any(b1 != 0.0):
        # device pipeline folds dw through relu; exact only for b1 == 0
        return _reference_numpy(
            batch_a, coords_a, feats_a, batch_b, coords_b, feats_b, W1, b1, W2, b2
        )

    in_maps, meta = _host_prep(
        batch_a, coords_a, batch_b, coords_b, feats_b, W1, b1, W2, b2
    )
    if in_maps is None:
        return _reference_numpy(
            batch_a, coords_a, feats_a, batch_b, coords_b, feats_b, W1, b1, W2, b2
        )

    nc = get_program()
    trace = bool(int(os.environ.get("KERNEL_TRACE", "0")))
    if trace:
        _ensure_ntff_hook()
    res = run_bass_kernel_spmd(
        nc, in_maps, core_ids=list(range(N_CORES)), trace=trace
    )
    kernel.last_results = res

    fused = np.zeros((len(batch_a), DF), dtype=np.float32)
    for g in range(N_CORES):
        a_idx, na, nb = meta[g]
        if na == 0:
            continue
        out_g = res.results[g]["fusedT"]  # [DF, NPAD]
        if nb == 0:
            # reference: dw=0 rows -> h=0 -> fused = 3*b2
            fused[a_idx] = 3.0 * b2
        else:
            fused[a_idx] = out_g[:, :na].T
    return np.concatenate([feats_a, fused], axis=1)


# revision 20
# speedup vs baseline: 1.0945x; 1.0496x over previous
"""Batched same-batch KNN (top-3) + fused MLP for Trainium2, 8 NeuronCores.

Strategy
--------
Host side (numpy, exact):
  * Stable-group rows of a and b by batch id. Batch g -> core g (B == 8 ==
    n_cores). Within a batch the original relative order is preserved, so
    the device's tie handling matches jax.lax.top_k.
  * Composite-distance trick: the device computes, per [Na_g, Nb_g] pair,
        m[i,j] = -(dist2(i,j) + j * 2^-11)
    via ONE K=8 fp16 matmul (4 concurrent PE row-groups). All distance
    terms are small integers split so every fp16 operand and every fp32
    partial sum is exact; the j-term rides in the LAST K row, so it is
    added to the settled integer distance. For dist2 < 8192 (the only
    region where the weight dw > 0) the composite is exactly
    representable in fp32, so top-3 selection, tie order (smallest j
    first, = jax), the gather index j, and dw are all exact.
Device side (per core, SPMD):
  * R = relu(feats_bg @ W1) in bf16 DRAM (b1 == 0 asserted host-side;
    rows 1664..2047 zeroed so don't-care indices stay harmless).
  * Per 128-row a-tile: K=8 matmul -> 2 PSUM chunks; DVE max8 per chunk
    straight from PSUM + an 16-wide merge max8 (the only DVE scans);
    dw = relu(0.5 + m/16384) (j-term shifts dw by <= 6e-5, negligible);
    j = uint32(-m*2048) & 2047; ONE dma_gather fetches all 3*128 R rows
    (wrapped int16 index layout built by a tiny DRAM bounce); the
    dw^2-weighted sum + transpose fuse into 6 bf16 PE matmuls against
    diag(dw2_k) accumulating in PSUM; then the bf16 @W2 (+3*b2) tail
    writes the fusedT output slab.
Outputs are scattered back to original row order on host; the feats_a
passthrough half of the concat is host-side assembly.
"""

import os
import numpy as np
import ml_dtypes

import concourse.bass as bass
import concourse.mybir as mybir
import concourse.tile as tile
from concourse import bacc
from concourse.bass import IndirectOffsetOnAxis
from concourse.bass_utils import run_bass_kernel_spmd
from concourse.masks import make_identity

P = 128
NPAD = 1664  # 13 * 128; covers per-batch row counts for Na=Nb=12288, B=8
NT = NPAD // P
DF = 256
TOPK = 3
FULL_SCALE = 128
RCLIP = 0.5
INV_SCALE2 = 1.0 / (FULL_SCALE * FULL_SCALE)
N_CORES = 8
HCH = NPAD // 2  # 832: distance PSUM chunk width
QCH = NPAD // 4  # 416: per-PE-row-group column span
RROWS = 2048  # R table rows (pow2 so j & 2047 is always in bounds)

_PROGRAM_CACHE = {}


def _build_program():
    """Build the SPMD Bass program (identical on all 8 cores)."""
    nc = bacc.Bacc("TRN2", target_bir_lowering=False, debug=False)
    f32 = mybir.dt.float32
    f16 = mybir.dt.float16
    bf16 = mybir.dt.bfloat16
    i16 = mybir.dt.int16
    u32 = mybir.dt.uint32

    uaT = nc.dram_tensor("uaT", [8, NPAD], f16, kind="ExternalInput").ap()
    vbT = nc.dram_tensor("vbT", [8, NPAD], f16, kind="ExternalInput").ap()
    fbTb = nc.dram_tensor("fbTb", [DF, NPAD], bf16, kind="ExternalInput").ap()
    w1b = nc.dram_tensor("w1b", [DF, DF], bf16, kind="ExternalInput").ap()
    w2b = nc.dram_tensor("w2b", [DF, DF], bf16, kind="ExternalInput").ap()
    b2c3 = nc.dram_tensor("b2c3", [P, 2], f32, kind="ExternalInput").ap()
    fusedT = nc.dram_tensor("fusedT", [DF, NPAD], f32, kind="ExternalOutput").ap()

    with tile.TileContext(nc) as tc:
        with (
            tc.tile_pool(name="const", bufs=1) as cpool,
            tc.tile_pool(name="dram", bufs=1, space="DRAM") as dpool_dram,
        ):
            # R-phase dependencies first: their loads gate rtab, which gates
            # every gather
            fbT0 = cpool.tile([P, NPAD], bf16)
            nc.sync.dma_start(fbT0[:], fbTb[0:P, :])
            fbT1 = cpool.tile([P, NPAD], bf16)
            nc.sync.dma_start(fbT1[:], fbTb[P : 2 * P, :])
            w1k0 = cpool.tile([P, DF], bf16)
            nc.sync.dma_start(w1k0[:], w1b[0:P, :])
            w1k1 = cpool.tile([P, DF], bf16)
            nc.sync.dma_start(w1k1[:], w1b[P : 2 * P, :])
            ua4 = cpool.tile([8, NPAD], f16)
            nc.sync.dma_start(ua4[:], uaT[:])
            vb4 = cpool.tile([8, NPAD], f16)
            nc.sync.dma_start(vb4[:], vbT[:])
            w2k0 = cpool.tile([P, DF], bf16)
            nc.sync.dma_start(w2k0[:], w2b[0:P, :])
            w2k1 = cpool.tile([P, DF], bf16)
            nc.sync.dma_start(w2k1[:], w2b[P : 2 * P, :])
            b2s = cpool.tile([P, 2], f32)
            nc.sync.dma_start(b2s[:], b2c3[:])
            identb = cpool.tile([P, P], bf16)
            make_identity(nc, identb[:])
            halfcol = cpool.tile([P, 1], f32)
            nc.gpsimd.memset(halfcol[:], RCLIP)
            zrow = cpool.tile([P, DF], bf16)
            nc.gpsimd.memset(zrow[:], 0.0)

            rtab = dpool_dram.tile([RROWS, DF], bf16)
            # zero the don't-care tail rows 1664..2047 right away
            for t in range(NT, RROWS // P):
                nc.sync.dma_start(rtab[bass.ts(t, P), :], zrow[:])

            # ---- Phase D pools (f_pool doubles as the R-phase MM1 PSUM so
            # the early top-k tiles can overlap the R phase)
            with (
                tc.tile_pool(name="dps", bufs=2, space="PSUM") as d_pool,
                tc.tile_pool(name="tps", bufs=2, space="PSUM") as t_pool,
                tc.tile_pool(name="fps", bufs=2, space="PSUM") as f_pool,
                tc.tile_pool(name="small", bufs=8) as s_pool,
                tc.tile_pool(name="diag", bufs=8) as diag_pool,
                tc.tile_pool(name="gat", bufs=8) as g_pool,
                tc.tile_pool(name="acc", bufs=2) as a_pool,
                tc.tile_pool(name="outp", bufs=2) as o_pool,
                tc.tile_pool(name="rsb", bufs=3) as r_pool,
            ):
                state = {}

                def r_phase():
                    rt = None
                    for t in range(NT):
                        sl = bass.ts(t, P)
                        pool = f_pool if t % 2 == 0 else t_pool
                        psR = pool.tile([P, DF], f32, tag="fus" if t % 2 == 0 else "accT")
                        # b1 asserted zero host-side (numpy fallback otherwise)
                        nc.tensor.matmul(
                            psR[:], lhsT=fbT0[:, sl], rhs=w1k0[:],
                            start=True, stop=False,
                        )
                        nc.tensor.matmul(
                            psR[:], lhsT=fbT1[:, sl], rhs=w1k1[:],
                            start=False, stop=True,
                        )
                        # stage 4 tiles of relu'd R per DMA: 13 -> 4 dispatches
                        if t % 4 == 0:
                            rt = r_pool.tile([P, 4 * DF], bf16)
                        nc.scalar.activation(
                            rt[:, (t % 4) * DF : (t % 4 + 1) * DF],
                            psR[:],
                            mybir.ActivationFunctionType.Relu,
                        )
                        if t % 4 == 3 or t == NT - 1:
                            n = t % 4 + 1
                            base = t - n + 1
                            nc.sync.dma_start(
                                rtab[base * P : (base + n) * P, :].rearrange(
                                    "(tt p) c -> p tt c", tt=n
                                ),
                                rt[:, : n * DF].rearrange(
                                    "p (tt c) -> p tt c", tt=n
                                ),
                            )


                def topk_and_gather(t):
                    sl = bass.ts(t, P)
                    vals = s_pool.tile([P, 16], f32, tag="vals")
                    for c in range(2):
                        dps = d_pool.tile([P, HCH], f32)
                        # two matmuls per chunk (512 + 320 cols), each within
                        # one PSUM bank; 4 PE row-groups across the two chunks
                        for l0, l1 in ((0, 512), (512, HCH)):
                            nc.tensor.matmul(
                                dps[:, l0:l1],
                                lhsT=ua4[:, sl],
                                rhs=vb4[:, c * HCH + l0 : c * HCH + l1],
                                start=True,
                                stop=True,
                            )
                        # top-8 of this chunk, straight from PSUM
                        nc.vector.max(out=vals[:, 8 * c : 8 * (c + 1)], in_=dps[:])
                    vals3 = s_pool.tile([P, 8], f32, tag="vals3")
                    nc.vector.max(out=vals3[:], in_=vals[:])
                    # dw = relu(0.5 + m/16384); j-term shifts it by <=6e-5
                    dw = s_pool.tile([P, TOPK], f32, tag="dw")
                    nc.scalar.activation(
                        dw[:],
                        vals3[:, 0:TOPK],
                        mybir.ActivationFunctionType.Relu,
                        bias=halfcol[:],
                        scale=INV_SCALE2,
                    )
                    # j = uint32(-m * 2048) & 2047 — issued before the diag
                    # builds so the gathers unblock as early as possible
                    c32 = s_pool.tile([P, TOPK], u32, tag="c32")
                    nc.scalar.mul(c32[:], vals3[:, 0:TOPK], -2048.0)
                    jand = s_pool.tile([P, TOPK], u32, tag="jand")
                    nc.vector.tensor_scalar(
                        out=jand[:],
                        in0=c32[:],
                        scalar1=2047,
                        scalar2=None,
                        op0=mybir.AluOpType.bitwise_and,
                    )
                    dw2 = s_pool.tile([P, TOPK], f32, tag="dw2")
                    nc.scalar.activation(
                        dw2[:], dw[:], mybir.ActivationFunctionType.Square
                    )
                    # diag_k = dw2[:, k] on the diagonal (bf16), PE rhs operands
                    diag = diag_pool.tile([P, TOPK * P], bf16)
                    for k in range(TOPK):
                        nc.scalar.mul(
                            diag[:, k * P : (k + 1) * P], identb[:], dw2[:, k : k + 1]
                        )
                    state[t] = (diag, jand)

                def gather_tile(t):
                    # 3 indirect gathers (SWDGE descriptors spread across all
                    # 16 DMA engines): g[p, k*DF:(k+1)*DF] = R[jand[p, k]].
                    # MUST be issued after the rtab writes in program order.
                    diag, jand = state.pop(t)
                    g = g_pool.tile([P, TOPK * DF], bf16)
                    for k in range(TOPK):
                        nc.gpsimd.indirect_dma_start(
                            out=g[:, k * DF : (k + 1) * DF],
                            out_offset=None,
                            in_=rtab[:],
                            in_offset=IndirectOffsetOnAxis(
                                ap=jand[:, k : k + 1], axis=0
                            ),
                        )
                    state[t] = (diag, g)

                def mlp_tile(t):
                    sl = bass.ts(t, P)
                    diag, g = state.pop(t)
                    # accT[i, p] = sum_k dw2_k[p] * g_k[p, i]  (scaled transpose,
                    # PSUM-accumulated over k; m indexes the two i-halves)
                    accT = t_pool.tile([P, DF], f32, tag="accT")
                    for m in range(2):
                        for k in range(TOPK):
                            nc.tensor.matmul(
                                accT[:, m * P : (m + 1) * P],
                                lhsT=g[:, k * DF + m * P : k * DF + (m + 1) * P],
                                rhs=diag[:, k * P : (k + 1) * P],
                                start=(k == 0),
                                stop=(k == TOPK - 1),
                            )
                    accTs = a_pool.tile([P, DF], bf16)
                    nc.scalar.copy(accTs[:], accT[:])
                    # fusedT[o, p] = sum_i W2[i, o] * accT[i, p]  (+ 3*b2[o])
                    fus = f_pool.tile([P, DF], f32, tag="fus")
                    for mo in range(2):
                        osl = slice(mo * P, (mo + 1) * P)
                        nc.tensor.matmul(
                            fus[:, osl],
                            lhsT=w2k0[:, osl],
                            rhs=accTs[:, 0:P],
                            start=True,
                            stop=False,
                        )
                        nc.tensor.matmul(
                            fus[:, osl],
                            lhsT=w2k1[:, osl],
                            rhs=accTs[:, P : 2 * P],
                            start=False,
                            stop=True,
                        )
                    oT = o_pool.tile([P, DF], f32)
                    for mo in range(2):
                        osl = slice(mo * P, (mo + 1) * P)
                        nc.vector.tensor_scalar(
                            out=oT[:, osl],
                            in0=fus[:, osl],
                            scalar1=b2s[:, mo : mo + 1],
                            scalar2=None,
                            op0=mybir.AluOpType.add,
                        )
                        nc.sync.dma_start(fusedT[osl, sl], oT[:, osl])

                SKEW = 3
                # R phase first: MM1 completion gates every gather, so no
                # other PE work may precede it
                r_phase()
                for t in range(NT + SKEW):
                    if t < NT:
                        topk_and_gather(t)
                        gather_tile(t)
                    if t >= SKEW:
                        mlp_tile(t - SKEW)
    nc.compile()
    return nc


def get_program():
    if "nc" not in _PROGRAM_CACHE:
        _PROGRAM_CACHE["nc"] = _build_program()
    return _PROGRAM_CACHE["nc"]


def _host_prep(batch_a, coords_a, batch_b, coords_b, feats_b, W1, b1, W2, b2):
    """Group by batch, build per-core input arrays. Returns (in_maps, meta)."""
    pa = np.argsort(batch_a, kind="stable")
    pb = np.argsort(batch_b, kind="stable")
    ca = np.bincount(batch_a, minlength=N_CORES)
    cb = np.bincount(batch_b, minlength=N_CORES)
    oa = np.concatenate([[0], np.cumsum(ca)])
    ob = np.concatenate([[0], np.cumsum(cb)])

    w1bf = W1.astype(ml_dtypes.bfloat16)
    w2bf = W2.astype(ml_dtypes.bfloat16)
    b2c3 = np.ascontiguousarray((3.0 * b2).astype(np.float32).reshape(2, P).T)
    jrow = -(np.arange(NPAD).astype(np.float64) * 2.0**-11)

    in_maps = []
    meta = []
    for g in range(N_CORES):
        a_idx = pa[oa[g] : oa[g + 1]]
        b_idx = pb[ob[g] : ob[g + 1]]
        na, nb = len(a_idx), len(b_idx)
        if na > NPAD or nb > NPAD or (0 < nb < TOPK):
            return None, None  # shapes outside the compiled envelope -> fallback
        xa = (coords_a[a_idx] // 16).astype(np.int64)
        xb = (coords_b[b_idx] // 16).astype(np.int64)
        if xa.size and (xa.min() < 0 or xa.max() > 127):
            return None, None  # outside the exact-f16 envelope -> fallback
        if xb.size and (xb.min() < 0 or xb.max() > 127):
            return None, None

        # composite-distance operands (all values exactly representable in f16)
        A = np.square(xa).sum(1)  # [na], <= 3*127^2 = 48387
        B = np.square(xb).sum(1)
        uaT = np.zeros((8, NPAD), dtype=np.float16)
        uaT[2, :] = 1.0
        uaT[3, :] = 1.0
        uaT[7, :] = 1.0
        if na > 0:
            uaT[0, :na] = -(A // 256)
            uaT[1, :na] = -(A % 256)
            uaT[4:7, :na] = (2 * xa).T
            if na < NPAD:
                uaT[:, na:] = uaT[:, :1]  # pad a-rows: copy of row 0, dropped
        vbT = np.zeros((8, NPAD), dtype=np.float16)
        vbT[0, :] = 256.0
        vbT[1, :] = 1.0
        vbT[2, :] = -256.0 * 200.0  # pad cols: dist2 ~ 51200, never in top-3
        vbT[7, :] = jrow
        if nb > 0:
            vbT[2, :nb] = -256.0 * (B // 256)
            vbT[3, :nb] = -(B % 256)
            vbT[4:7, :nb] = xb.T

        fbT = np.zeros((DF, NPAD), dtype=ml_dtypes.bfloat16)
        if nb > 0:
            fbT[:, :nb] = feats_b[b_idx].astype(ml_dtypes.bfloat16).T

        in_maps.append(
            {
                "uaT": uaT,
                "vbT": vbT,
                "fbTb": fbT,
                "w1b": w1bf,
                "w2b": w2bf,
                "b2c3": b2c3,
            }
        )
        meta.append((a_idx, na, nb))
    return in_maps, meta


def _reference_numpy(batch_a, coords_a, feats_a, batch_b, coords_b, feats_b,
                     W1, b1, W2, b2):
    """Exact numpy fallback (mirrors reference.py) for out-of-envelope data."""
    xa = (coords_a // 16).astype(np.float32)
    xb = (coords_b // 16).astype(np.float32)
    d = (
        np.square(xa).sum(1)[:, None]
        + np.square(xb).sum(1)[None, :]
        - 2.0 * (xa @ xb.T)
    )
    d = np.clip(d, 0.0, None) / (FULL_SCALE**2)
    same = batch_a[:, None] == batch_b[None, :]
    d = np.where(same, d, np.inf)
    idx = np.argsort(d, axis=1, kind="stable")[:, :TOPK]
    dv = np.take_along_axis(d, idx, axis=1)
    dwt = RCLIP - np.clip(dv, 0.0, RCLIP)
    b_f = feats_b[idx] * dwt[..., None]
    h = np.maximum(b_f @ W1 + b1, 0.0) * dwt[..., None]
    fused = (h @ W2 + b2).sum(axis=1)
    return np.concatenate([feats_a, fused], axis=1).astype(np.float32)


def _ensure_ntff_hook():
    """Install the axon NTFF profile hook (missing antenv.axon_hooks shim)."""
    import sys
    import types

    if "antenv.axon_hooks" in sys.modules:
        return
    try:
        from trn_agent_boot.trn_boot import _ntff_profile_via_ctypes

        hook = _ntff_profile_via_ctypes("/opt/axon/libaxon_pjrt.so")
    except Exception:
        hook = None
    mod = types.ModuleType("antenv.axon_hooks")
    _state = {"hook": hook}
    mod.get_axon_ntff_profile_hook = lambda: _state["hook"]

    def _set(h):
        _state["hook"] = h

    mod.set_axon_ntff_profile_hook = _set
    sys.modules["antenv.axon_hooks"] = mod


def kernel(batch_a, coords_a, feats_a, batch_b, coords_b, feats_b, W1, b1, W2, b2):
    batch_a = np.asarray(batch_a)
    coords_a = np.asarray(coords_a)
    feats_a = np.asarray(feats_a, dtype=np.float32)
    batch_b = np.asarray(batch_b)
    coords_b = np.asarray(coords_b)
    feats_b = np.asarray(feats_b, dtype=np.float32)
    W1 = np.asarray(W1, dtype=np.float32)
    b1 = np.asarray(b1, dtype=np.float32)
    W2 = np.asarray(W2, dtype=np.float32)
    b2 = np.asarray(b2, dtype=np.float32)

    if np.any(b1 != 0.0):
        # device pipeline folds dw through relu; exact only for b1 == 0
        return _reference_numpy(
            batch_a, coords_a, feats_a, batch_b, coords_b, feats_b, W1, b1, W2, b2
        )

    in_maps, meta = _host_prep(
        batch_a, coords_a, batch_b, coords_b, feats_b, W1, b1, W2, b2
    )
    if in_maps is None:
        return _reference_numpy(
            batch_a, coords_a, feats_a, batch_b, coords_b, feats_b, W1, b1, W2, b2
        )

    nc = get_program()
    trace = bool(int(os.environ.get("KERNEL_TRACE", "0")))
    if trace:
        _ensure_ntff_hook()
    res = run_bass_kernel_spmd(
        nc, in_maps, core_ids=list(range(N_CORES)), trace=trace
    )
    kernel.last_results = res

    fused = np.zeros((len(batch_a), DF), dtype=np.float32)
    for g in range(N_CORES):
        a_idx, na, nb = meta[g]
        if na == 0:
            continue
        out_g = res.results[g]["fusedT"]  # [DF, NPAD]
        if nb == 0:
            # reference: dw=0 rows -> h=0 -> fused = 3*b2
            fused[a_idx] = 3.0 * b2
        else:
            fused[a_idx] = out_g[:, :na].T
    return np.concatenate([feats_a, fused], axis=1)
